# revision 11
# baseline (speedup 1.0000x reference)
"""Trainium2 Bass kernel for nn_K_Rectify (gnn message passing, idw + rmsnorm).

Reference computation (B=128, NTOT=129, N=128, GS=16, C=384):
    x   = f[:, 1:, :]                         # [B, N, C]
    nf  = x.reshape(B*N, C)[idx]              # [B, N, GS, C] gather (global flat idx)
    w   = 1/(dist+eps); w /= w.sum(-1)        # idw weights
    sf  = sum_g w * (nf - x) = (sum_g w*nf) - x    (weights sum to 1)
    out = (rf[1:] + x) + rmsnorm(sf) * knorm_w
    cat cls token back on.

Sharding: data-parallel over batch B across 8 cores (16 batches / core).
idx values index the full flattened [B*N] table, so the gather source
table is replicated to every core; everything else is sharded.

The random-row gather dominates; SWDGE descriptor cost is ~2.2 ns fixed
+ ~2 ns/KB, so the gather table is stored fp8e4 padded to 512 B rows
(measured 100.6 us for the 32768-row gather vs 173 us in f32). The
weighted neighbor sum runs entirely on the PE as mixed-precision
matmuls (bf16 diag-weight lhsT x fp8 neighbor rhs -> f32 PSUM), which
hardware-probes exact. The residual path (x, x+rf, output) stays f32;
rmsnorm in f32. idw weights + identity + x+rf are host-precomputed.
"""

import sys

sys.path.insert(0, "/opt/trn_rl_repo")

import numpy as np

import concourse.bacc as bacc
import concourse.mybir as mybir
import concourse.tile as tile
from concourse import bass
from concourse.bass_utils import run_bass_kernel_spmd

B, NTOT, N, GS, C = 128, 129, 128, 16, 384
EPS = 0.05
RMS_EPS = 1e-6
NCORES = 8
SHB = B // NCORES            # batches per core (16)
PTS = SHB * N                # points per core (2048)
P = 128                      # partitions
TILES = PTS // P             # point-tiles per core (16)
ROWS = B * N                 # gather table rows (16384)
RPAD = 512                   # fp8 row padded to 512 B (elem_size % 256 == 0)

F32 = mybir.dt.float32
BF16 = mybir.dt.bfloat16
FP8 = mybir.dt.float8e4
I16 = mybir.dt.int16
NP_BF16 = mybir.dt.np(BF16)
NP_FP8 = mybir.dt.np(FP8)

_CACHE = {}


def _build(knw_is_ones=True):
    # 64 KB dynamic-DMA scratch -> 4096-descriptor SWDGE ring so several
    # 1024-descriptor gathers can be in flight.
    nc = bacc.Bacc(
        "TRN2", target_bir_lowering=False, debug=False,
        dynamic_dma_scratch_size=65536, num_swdge_queues=4,
    )

    xall = nc.dram_tensor("xall", [ROWS, RPAD], FP8, kind="ExternalInput")
    xs = nc.dram_tensor("xs", [P, TILES, C], F32, kind="ExternalInput")
    fbase = nc.dram_tensor("fbase", [P, TILES, C], F32, kind="ExternalInput")
    wnb = nc.dram_tensor("wnb", [P, TILES, GS], BF16, kind="ExternalInput")
    idxw = nc.dram_tensor("idxw", [P, PTS], I16, kind="ExternalInput")
    identw = nc.dram_tensor("identw", [P, P], BF16, kind="ExternalInput")
    knw = nc.dram_tensor("knw", [P, C], BF16, kind="ExternalInput")
    out = nc.dram_tensor("out", [P, TILES, C], F32, kind="ExternalOutput")

    with tile.TileContext(nc) as tc:
        with (
            tc.tile_pool(name="consts", bufs=1) as cpool,
            tc.tile_pool(name="gbuf", bufs=4) as gpool,
            tc.tile_pool(name="work", bufs=3) as wpool,
            tc.tile_pool(name="small", bufs=3) as spool,
            tc.tile_pool(name="psum", bufs=4, space="PSUM") as ppool,
        ):
            # Warmup: the first dma_gather pays a fixed ~8-13 us SWDGE
            # ucode/ring init. Fire a throwaway 128-index gather (indices
            # memset to row 0) as the very first gpsimd work so the init
            # overlaps the idx/const DMAs instead of delaying the stream.
            warm_idx = cpool.tile([P, 8], I16)
            nc.gpsimd.memset(warm_idx[:], 0)
            warm_out = cpool.tile([P, 1, RPAD], FP8)
            nc.gpsimd.dma_gather(
                out_ap=warm_out[:],
                in_ap=xall[:],
                idxs_ap=warm_idx[:],
                num_idxs=P,
                num_idxs_reg=P,
                elem_size=RPAD,
                queue_num=0,
            )

            # idx next: the first real gather depends only on it.
            idx_t = cpool.tile([P, PTS], I16)
            nc.sync.dma_start(idx_t[:], idxw[:])

            half = P * GS // 2
            LOOKAHEAD = 2

            def issue_gathers(j, nbr):
                # nbr[p, g, :] = xall[idx[j, p, g], :]; two 1024-index
                # gathers (>1024 per instruction faults the SWDGE ucode),
                # round-robined over the 4 SWDGE queues.
                for h in range(2):
                    nc.gpsimd.dma_gather(
                        out_ap=nbr[:, h * (GS // 2) : (h + 1) * (GS // 2), :],
                        in_ap=xall[:],
                        idxs_ap=idx_t[:, j * P + h * (half // 16) : j * P + (h + 1) * (half // 16)],
                        num_idxs=half,
                        num_idxs_reg=half,
                        elem_size=RPAD,
                        queue_num=(2 * j + h) % 4,
                    )

            nbr_tiles = {}
            for j in range(LOOKAHEAD):
                nbr_tiles[j] = gpool.tile([P, GS, RPAD], FP8, tag="nbr", name=f"nbr{j}")
                issue_gathers(j, nbr_tiles[j])

            wnb_t = cpool.tile([P, TILES, GS], BF16)
            nc.sync.dma_start(wnb_t[:], wnb[:])
            ident = cpool.tile([P, P], BF16)
            nc.sync.dma_start(ident[:], identw[:])
            knw_t = cpool.tile([P, C], BF16)
            nc.sync.dma_start(knw_t[:], knw[:])
            epsb = cpool.tile([P, 1], F32)
            nc.vector.memset(epsb[:], RMS_EPS)
            ident_b = ident[:].rearrange("p (x c) -> p x c", x=1).to_broadcast(
                [P, GS, P]
            )

            xs_t = cpool.tile([P, TILES, C], F32)
            nc.sync.dma_start(xs_t[:], xs[:])
            fb_t = cpool.tile([P, TILES, C], F32)
            nc.sync.dma_start(fb_t[:], fbase[:])

            for j in range(TILES):
                if j + LOOKAHEAD < TILES:
                    nbr_tiles[j + LOOKAHEAD] = gpool.tile(
                        [P, GS, RPAD], FP8, tag="nbr", name=f"nbr{j + LOOKAHEAD}"
                    )
                    issue_gathers(j + LOOKAHEAD, nbr_tiles[j + LOOKAHEAD])
                nbr = nbr_tiles.pop(j)

                # weighted neighbor sum entirely on the PE:
                # acc = sum_g diag(w_g) @ nbr_g  (bf16 lhsT x fp8 rhs)
                dmat = wpool.tile([P, GS, P], BF16, tag="dmat")
                nc.vector.tensor_tensor(
                    out=dmat[:],
                    in0=ident_b,
                    in1=wnb_t[:, j, :].to_broadcast([P, GS, P]),
                    op=mybir.AluOpType.mult,
                )
                acc_p = ppool.tile([P, C], F32, tag="acc")
                for g in range(GS):
                    nc.tensor.matmul(
                        out=acc_p[:],
                        lhsT=dmat[:, g, :],
                        rhs=nbr[:, g, :C],
                        start=(g == 0),
                        stop=(g == GS - 1),
                    )

                # sf = acc - x
                sf = wpool.tile([P, C], F32, tag="sf")
                nc.vector.tensor_tensor(
                    out=sf[:], in0=acc_p[:], in1=xs_t[:, j, :],
                    op=mybir.AluOpType.subtract,
                )

                # rmsnorm: rr = 1/sqrt(mean(sf^2) + eps)
                sq = wpool.tile([P, C], BF16, tag="sq")
                ssq = spool.tile([P, 1], F32, tag="ssq")
                nc.scalar.activation(
                    out=sq[:], in_=sf[:],
                    func=mybir.ActivationFunctionType.Square,
                    accum_out=ssq[:],
                )
                rms = spool.tile([P, 1], F32, tag="rms")
                nc.scalar.activation(
                    out=rms[:], in_=ssq[:],
                    func=mybir.ActivationFunctionType.Sqrt,
                    scale=1.0 / C, bias=epsb[:, :1],
                )
                rr = spool.tile([P, 1], F32, tag="rr")
                nc.vector.reciprocal(rr[:], rms[:])

                # normed = sf * rr (per-partition scale on ACT)
                nt = wpool.tile([P, C], F32, tag="nt")
                nc.scalar.activation(
                    out=nt[:], in_=sf[:],
                    func=mybir.ActivationFunctionType.Copy,
                    scale=rr[:, :1],
                )
                if not knw_is_ones:
                    nc.vector.tensor_tensor(
                        out=nt[:], in0=nt[:], in1=knw_t[:], op=mybir.AluOpType.mult
                    )

                # out = (x + rf) + normed   (x+rf host-precomputed)
                ot = wpool.tile([P, C], F32, tag="ot")
                nc.vector.tensor_tensor(
                    out=ot[:], in0=fb_t[:, j, :], in1=nt[:],
                    op=mybir.AluOpType.add,
                )
                nc.sync.dma_start(out[:, j, :], ot[:])

    nc.compile()
    return nc


def _get_nc(knw_is_ones=True):
    key = ("nc", knw_is_ones)
    if key not in _CACHE:
        _CACHE[key] = _build(knw_is_ones)
    return _CACHE[key]


def _wrap_idx(idx_core):
    """[PTS, GS] int -> [P, PTS] int16 wrapped layout for dma_gather.

    For tile j, half h (neighbors 8h..8h+7), gather-list position i
    (0..1023) lands in dst[i % 128, i // 128]; we want
    dst[p, g_h] = idx[j*128+p, 8h+g_h], so list[i] = blk[i % 128, 8h + i//128].
    The HW reads list[i] from idxs[i % 16, i // 16] over 16 partitions,
    and that [16, S] block must be replicated to all 128 partitions
    (each Q7 core reads its own copy).
    """
    out = np.zeros((P, PTS), np.int16)
    half = P * GS // 2                               # 1024
    S = half // 16                                   # 64
    for j in range(TILES):
        blk = idx_core[j * P : (j + 1) * P]          # [128, 16]
        for h in range(2):
            lst = blk[:, h * (GS // 2) : (h + 1) * (GS // 2)].T.reshape(-1)
            wrapped = lst.reshape(S, 16).T           # [16, 64]
            col = j * P + h * S
            out[:, col : col + S] = np.tile(wrapped, (P // 16, 1))
    return out


def _tilewise(a):
    """[PTS, C...] -> [P, TILES, C...] with [p, j] = row j*128+p."""
    return np.ascontiguousarray(
        a.reshape(TILES, P, *a.shape[1:]).transpose(1, 0, *range(2, a.ndim + 1))
    )


def _make_in_maps(inputs):
    f = np.asarray(inputs["f"], dtype=np.float32)
    distance = np.asarray(inputs["distance"], dtype=np.float32)
    rf = np.asarray(inputs["rf"], dtype=np.float32)
    knorm_w = np.asarray(inputs["knorm_w"], dtype=np.float32)
    idx_np = np.asarray(inputs["idx"]).astype(np.int64)

    x = f[:, NTOT - N :, :].reshape(ROWS, C)
    x8 = np.zeros((ROWS, RPAD), NP_FP8)
    x8[:, :C] = x.astype(NP_FP8)
    rfx = rf[NTOT - N :][:P]                         # [128, C] per-point bias
    knw_np = np.ascontiguousarray(
        np.broadcast_to(knorm_w.astype(NP_BF16), (P, C)).copy()
    )
    ident_np = np.zeros((P, P), NP_BF16)
    np.fill_diagonal(ident_np, 1.0)

    # idw weights on host: wn[p, j, g]
    u = 1.0 / (distance + EPS)
    wn = (u / u.sum(-1, keepdims=True)).astype(np.float32)

    in_maps = []
    for c in range(NCORES):
        bs = slice(c * SHB, (c + 1) * SHB)
        idx_core = idx_np[bs].reshape(PTS, GS)
        x_core = x[c * PTS : (c + 1) * PTS]
        fb_core = (x_core.reshape(PTS // N, N, C) + rfx).reshape(PTS, C)
        in_maps.append(
            {
                "xall": x8,
                "xs": _tilewise(x_core.astype(np.float32)),
                "fbase": _tilewise(fb_core.astype(np.float32)),
                "wnb": _tilewise(
                    wn[bs].reshape(PTS, GS).astype(NP_BF16)
                ),
                "idxw": _wrap_idx(idx_core),
                "identw": ident_np,
                "knw": knw_np,
            }
        )
    return in_maps


def kernel(f, distance, rf, knorm_w, idx, **_unused):
    f = np.asarray(f, dtype=np.float32)
    in_maps = _make_in_maps(
        {"f": f, "distance": distance, "rf": rf, "knorm_w": knorm_w, "idx": idx}
    )

    nc = _get_nc(bool(np.all(np.asarray(knorm_w) == 1.0)))
    res = run_bass_kernel_spmd(nc, in_maps, list(range(NCORES)))

    out = np.empty((B, NTOT, C), np.float32)
    out[:, : NTOT - N, :] = f[:, : NTOT - N, :]
    for c in range(NCORES):
        body = res.results[c]["out"]                 # [P, TILES, C]
        out[c * SHB : (c + 1) * SHB, NTOT - N :, :] = (
            body.transpose(1, 0, 2).reshape(SHB, N, C)
        )
    return out


# revision 12
# speedup vs baseline: 1.0608x; 1.0608x over previous
"""Trainium2 Bass kernel for nn_K_Rectify (gnn message passing, idw + rmsnorm).

Reference computation (B=128, NTOT=129, N=128, GS=16, C=384):
    x   = f[:, 1:, :]                         # [B, N, C]
    nf  = x.reshape(B*N, C)[idx]              # [B, N, GS, C] gather (global flat idx)
    w   = 1/(dist+eps); w /= w.sum(-1)        # idw weights
    sf  = sum_g w * (nf - x) = (sum_g w*nf) - x    (weights sum to 1)
    out = (rf[1:] + x) + rmsnorm(sf) * knorm_w
    cat cls token back on.

Sharding: data-parallel over batch B across 8 cores (16 batches / core).
idx values index the full flattened [B*N] table, so the gather source
table is replicated to every core; everything else is sharded.

The random-row gather dominates; SWDGE descriptor cost is ~2.2 ns fixed
+ ~2 ns/KB, so the gather table is stored fp8e4 padded to 512 B rows
(measured 100.6 us for the 32768-row gather vs 173 us in f32). The
weighted neighbor sum runs entirely on the PE as mixed-precision
matmuls (bf16 diag-weight lhsT x fp8 neighbor rhs -> f32 PSUM), which
hardware-probes exact. The residual path (x, x+rf, output) stays f32;
rmsnorm in f32. idw weights + identity + x+rf are host-precomputed.
"""

import sys

sys.path.insert(0, "/opt/trn_rl_repo")

import numpy as np

import concourse.bacc as bacc
import concourse.mybir as mybir
import concourse.tile as tile
from concourse import bass
from concourse.bass_utils import run_bass_kernel_spmd

B, NTOT, N, GS, C = 128, 129, 128, 16, 384
EPS = 0.05
RMS_EPS = 1e-6
NCORES = 8
SHB = B // NCORES            # batches per core (16)
PTS = SHB * N                # points per core (2048)
P = 128                      # partitions
TILES = PTS // P             # point-tiles per core (16)
ROWS = B * N                 # gather table rows (16384)
RPAD = 512                   # fp8 row padded to 512 B (elem_size % 256 == 0)

F32 = mybir.dt.float32
BF16 = mybir.dt.bfloat16
FP8 = mybir.dt.float8e4
I16 = mybir.dt.int16
NP_BF16 = mybir.dt.np(BF16)
NP_FP8 = mybir.dt.np(FP8)

_CACHE = {}


def _build(knw_is_ones=True):
    # 64 KB dynamic-DMA scratch -> 4096-descriptor SWDGE ring so several
    # 1024-descriptor gathers can be in flight.
    nc = bacc.Bacc(
        "TRN2", target_bir_lowering=False, debug=False,
        dynamic_dma_scratch_size=65536, num_swdge_queues=4,
    )

    xall = nc.dram_tensor("xall", [ROWS, RPAD], FP8, kind="ExternalInput")
    xs = nc.dram_tensor("xs", [P, TILES, C], F32, kind="ExternalInput")
    fbase = nc.dram_tensor("fbase", [P, TILES, C], F32, kind="ExternalInput")
    wnb = nc.dram_tensor("wnb", [P, TILES, GS], BF16, kind="ExternalInput")
    idxw = nc.dram_tensor("idxw", [P, PTS], I16, kind="ExternalInput")
    identw = nc.dram_tensor("identw", [P, P], BF16, kind="ExternalInput")
    knw = nc.dram_tensor("knw", [P, C], BF16, kind="ExternalInput")
    out = nc.dram_tensor("out", [P, TILES, C], F32, kind="ExternalOutput")

    with tile.TileContext(nc) as tc:
        with (
            tc.tile_pool(name="consts", bufs=1) as cpool,
            tc.tile_pool(name="gbuf", bufs=4) as gpool,
            tc.tile_pool(name="work", bufs=3) as wpool,
            tc.tile_pool(name="small", bufs=3) as spool,
            tc.tile_pool(name="psum", bufs=4, space="PSUM") as ppool,
        ):
            # idx first: the first gather depends only on it.
            idx_t = cpool.tile([P, PTS], I16)
            nc.sync.dma_start(idx_t[:], idxw[:])

            half = P * GS // 2
            LOOKAHEAD = 2

            def issue_gathers(j, nbr):
                # nbr[p, g, :] = xall[idx[j, p, g], :]; two 1024-index
                # gathers (>1024 per instruction faults the SWDGE ucode),
                # round-robined over the 4 SWDGE queues.
                for h in range(2):
                    nc.gpsimd.dma_gather(
                        out_ap=nbr[:, h * (GS // 2) : (h + 1) * (GS // 2), :],
                        in_ap=xall[:],
                        idxs_ap=idx_t[:, j * P + h * (half // 16) : j * P + (h + 1) * (half // 16)],
                        num_idxs=half,
                        num_idxs_reg=half,
                        elem_size=RPAD,
                        queue_num=(2 * j + h) % 4,
                    )

            nbr_tiles = {}
            for j in range(LOOKAHEAD):
                nbr_tiles[j] = gpool.tile([P, GS, RPAD], FP8, tag="nbr", name=f"nbr{j}")
                issue_gathers(j, nbr_tiles[j])

            wnb_t = cpool.tile([P, TILES, GS], BF16)
            nc.sync.dma_start(wnb_t[:], wnb[:])
            ident = cpool.tile([P, P], BF16)
            nc.sync.dma_start(ident[:], identw[:])
            knw_t = cpool.tile([P, C], BF16)
            nc.sync.dma_start(knw_t[:], knw[:])
            epsb = cpool.tile([P, 1], F32)
            nc.vector.memset(epsb[:], RMS_EPS)
            ident_b = ident[:].rearrange("p (x c) -> p x c", x=1).to_broadcast(
                [P, GS, P]
            )

            xs_t = cpool.tile([P, TILES, C], F32)
            nc.sync.dma_start(xs_t[:], xs[:])
            fb_t = cpool.tile([P, TILES, C], F32)
            nc.sync.dma_start(fb_t[:], fbase[:])

            for j in range(TILES):
                if j + LOOKAHEAD < TILES:
                    nbr_tiles[j + LOOKAHEAD] = gpool.tile(
                        [P, GS, RPAD], FP8, tag="nbr", name=f"nbr{j + LOOKAHEAD}"
                    )
                    issue_gathers(j + LOOKAHEAD, nbr_tiles[j + LOOKAHEAD])
                nbr = nbr_tiles.pop(j)

                # weighted neighbor sum entirely on the PE:
                # acc = sum_g diag(w_g) @ nbr_g  (bf16 lhsT x fp8 rhs)
                dmat = wpool.tile([P, GS, P], BF16, tag="dmat")
                nc.vector.tensor_tensor(
                    out=dmat[:],
                    in0=ident_b,
                    in1=wnb_t[:, j, :].to_broadcast([P, GS, P]),
                    op=mybir.AluOpType.mult,
                )
                acc_p = ppool.tile([P, C], F32, tag="acc")
                for g in range(GS):
                    nc.tensor.matmul(
                        out=acc_p[:],
                        lhsT=dmat[:, g, :],
                        rhs=nbr[:, g, :C],
                        start=(g == 0),
                        stop=(g == GS - 1),
                    )

                # sf = acc - x
                sf = wpool.tile([P, C], F32, tag="sf")
                nc.vector.tensor_tensor(
                    out=sf[:], in0=acc_p[:], in1=xs_t[:, j, :],
                    op=mybir.AluOpType.subtract,
                )

                # rmsnorm: rr = 1/sqrt(mean(sf^2) + eps)
                sq = wpool.tile([P, C], BF16, tag="sq")
                ssq = spool.tile([P, 1], F32, tag="ssq")
                nc.scalar.activation(
                    out=sq[:], in_=sf[:],
                    func=mybir.ActivationFunctionType.Square,
                    accum_out=ssq[:],
                )
                rms = spool.tile([P, 1], F32, tag="rms")
                nc.scalar.activation(
                    out=rms[:], in_=ssq[:],
                    func=mybir.ActivationFunctionType.Sqrt,
                    scale=1.0 / C, bias=epsb[:, :1],
                )
                rr = spool.tile([P, 1], F32, tag="rr")
                nc.vector.reciprocal(rr[:], rms[:])

                # normed = sf * rr (per-partition scale on ACT)
                nt = wpool.tile([P, C], F32, tag="nt")
                nc.scalar.activation(
                    out=nt[:], in_=sf[:],
                    func=mybir.ActivationFunctionType.Copy,
                    scale=rr[:, :1],
                )
                if not knw_is_ones:
                    nc.vector.tensor_tensor(
                        out=nt[:], in0=nt[:], in1=knw_t[:], op=mybir.AluOpType.mult
                    )

                # out = (x + rf) + normed   (x+rf host-precomputed)
                ot = wpool.tile([P, C], F32, tag="ot")
                nc.vector.tensor_tensor(
                    out=ot[:], in0=fb_t[:, j, :], in1=nt[:],
                    op=mybir.AluOpType.add,
                )
                nc.sync.dma_start(out[:, j, :], ot[:])

    nc.compile()
    return nc


def _get_nc(knw_is_ones=True):
    key = ("nc", knw_is_ones)
    if key not in _CACHE:
        _CACHE[key] = _build(knw_is_ones)
    return _CACHE[key]


def _wrap_idx(idx_core):
    """[PTS, GS] int -> [P, PTS] int16 wrapped layout for dma_gather.

    For tile j, half h (neighbors 8h..8h+7), gather-list position i
    (0..1023) lands in dst[i % 128, i // 128]; we want
    dst[p, g_h] = idx[j*128+p, 8h+g_h], so list[i] = blk[i % 128, 8h + i//128].
    The HW reads list[i] from idxs[i % 16, i // 16] over 16 partitions,
    and that [16, S] block must be replicated to all 128 partitions
    (each Q7 core reads its own copy).
    """
    out = np.zeros((P, PTS), np.int16)
    half = P * GS // 2                               # 1024
    S = half // 16                                   # 64
    for j in range(TILES):
        blk = idx_core[j * P : (j + 1) * P]          # [128, 16]
        for h in range(2):
            lst = blk[:, h * (GS // 2) : (h + 1) * (GS // 2)].T.reshape(-1)
            wrapped = lst.reshape(S, 16).T           # [16, 64]
            col = j * P + h * S
            out[:, col : col + S] = np.tile(wrapped, (P // 16, 1))
    return out


def _tilewise(a):
    """[PTS, C...] -> [P, TILES, C...] with [p, j] = row j*128+p."""
    return np.ascontiguousarray(
        a.reshape(TILES, P, *a.shape[1:]).transpose(1, 0, *range(2, a.ndim + 1))
    )


def _make_in_maps(inputs):
    f = np.asarray(inputs["f"], dtype=np.float32)
    distance = np.asarray(inputs["distance"], dtype=np.float32)
    rf = np.asarray(inputs["rf"], dtype=np.float32)
    knorm_w = np.asarray(inputs["knorm_w"], dtype=np.float32)
    idx_np = np.asarray(inputs["idx"]).astype(np.int64)

    x = f[:, NTOT - N :, :].reshape(ROWS, C)
    x8 = np.zeros((ROWS, RPAD), NP_FP8)
    x8[:, :C] = x.astype(NP_FP8)
    rfx = rf[NTOT - N :][:P]                         # [128, C] per-point bias
    knw_np = np.ascontiguousarray(
        np.broadcast_to(knorm_w.astype(NP_BF16), (P, C)).copy()
    )
    ident_np = np.zeros((P, P), NP_BF16)
    np.fill_diagonal(ident_np, 1.0)

    # idw weights on host: wn[p, j, g]
    u = 1.0 / (distance + EPS)
    wn = (u / u.sum(-1, keepdims=True)).astype(np.float32)

    in_maps = []
    for c in range(NCORES):
        bs = slice(c * SHB, (c + 1) * SHB)
        idx_core = idx_np[bs].reshape(PTS, GS)
        x_core = x[c * PTS : (c + 1) * PTS]
        fb_core = (x_core.reshape(PTS // N, N, C) + rfx).reshape(PTS, C)
        in_maps.append(
            {
                "xall": x8,
                "xs": _tilewise(x_core.astype(np.float32)),
                "fbase": _tilewise(fb_core.astype(np.float32)),
                "wnb": _tilewise(
                    wn[bs].reshape(PTS, GS).astype(NP_BF16)
                ),
                "idxw": _wrap_idx(idx_core),
                "identw": ident_np,
                "knw": knw_np,
            }
        )
    return in_maps


def kernel(f, distance, rf, knorm_w, idx, **_unused):
    f = np.asarray(f, dtype=np.float32)
    in_maps = _make_in_maps(
        {"f": f, "distance": distance, "rf": rf, "knorm_w": knorm_w, "idx": idx}
    )

    nc = _get_nc(bool(np.all(np.asarray(knorm_w) == 1.0)))
    res = run_bass_kernel_spmd(nc, in_maps, list(range(NCORES)))

    out = np.empty((B, NTOT, C), np.float32)
    out[:, : NTOT - N, :] = f[:, : NTOT - N, :]
    for c in range(NCORES):
        body = res.results[c]["out"]                 # [P, TILES, C]
        out[c * SHB : (c + 1) * SHB, NTOT - N :, :] = (
            body.transpose(1, 0, 2).reshape(SHB, N, C)
        )
    return out
